# revision 1
# baseline (speedup 1.0000x reference)
"""Trainium2 Bass kernel for the generalized filtered pairwise loss.

Math (reference semantics, N=2048 examples, L=128 positions, p in {1,2}):
  d = y_true - y_pred;  f = 1{|y_diff| <= 2};  m = d*f;  h = m^2
  lag-0 term:   sum_{n,i} W0[i,0]*|m_i| + W1[i,0]*h_i
  lag-k term (j=i+k<L, k>0), with B_p[i,j] = W_p[i, j-i]:
    p=1: sum_{n,i<j} B0[i,j] * |m_i f_j - f_i m_j|        (pairwise, needs abs)
    p=2: <B1, H^T F + F^T H - 2 M^T M>                     (factors into matmuls)
  loss = (sum of terms) / L / (N * mean(f))

Device strategy (8 cores, data-parallel over examples, 256/core):
  - per example e: X_e = m_e f_e^T - f_e m_e^T via one K=2 TensorE matmul
    (operands packed in 2-partition flat tiles -> base partition 0)
  - consume via relu identity (X antisymmetric => sum B0u.*|X| equals
    sum (B0u+B0u^T).*relu(X)): fused DVE scalar_tensor_tensor
    (max 0, then * Bs, accum per partition); some tiles offloaded through
    ACT-Relu->bf16 so their DVE pass runs at 2x
  - p=2 + lag-0 + sum(f) reductions via a handful of K=128 matmuls
  - small per-core partials DMA'd out; host combines in float64
"""

import os
import numpy as np
from contextlib import ExitStack

N, L = 2048, 128
NCORES = 8
NPC = N // NCORES            # 256 examples per core
NCH = 2                      # chunks of 128 examples
EX_PER_TILE = 16             # examples per PSUM X-tile (128 x 2048 = 4 banks)
NTILES = NPC // EX_PER_TILE  # 16
TILES_PER_CH = NTILES // NCH
FGV = 2.0

_STATE: dict = {}


def _patch_bir_wait_split():
    """Stock walrus rejects instructions with >1 sync-wait ('Too many sync
    wait commands'). Rewrite the BIR before compiling: for any instruction
    carrying k>1 waits, hoist k-1 of them onto single-wait NOPs inserted
    immediately before it on the same engine (identical semantics: the
    engine blocks on each wait in sequence before issuing the op)."""
    import json
    import concourse.bass_utils as bu
    import concourse.bass2jax as b2j

    if getattr(bu, "_wait_split_patched", False):
        return
    orig = bu.compile_bir_kernel

    def _split(bir_str):
        d = json.loads(bir_str)
        changed = False
        ctr = 0
        for fn in d.get("functions", []):
            for bb in fn.get("blocks", []):
                out = []
                for inst in bb.get("instructions", []):
                    si = inst.get("sync_info")
                    waits = (si or {}).get("on_wait") or []
                    if len(waits) > 1:
                        changed = True
                        for w in waits[:-1]:
                            ctr += 1
                            out.append({
                                "debug": inst.get("debug", 0),
                                "engine": inst["engine"],
                                "ins": [], "outs": [],
                                "name": f"{inst['name']}-ws{ctr}",
                                "opcode": "NoOp",
                                "sync_info": {"on_update": [], "on_wait": [w]},
                                "text_hint": "wait_split",
                            })
                        si["on_wait"] = [waits[-1]]
                    out.append(inst)
                bb["instructions"] = out
        if not changed:
            return bir_str
        return json.dumps(d).encode()

    def wrapper(bir_str, *args, **kwargs):
        return orig(_split(bir_str), *args, **kwargs)

    bu.compile_bir_kernel = wrapper
    b2j.compile_bir_kernel = wrapper
    bu._wait_split_patched = True


def _build_state():
    import concourse.bass as bass
    import concourse.tile as tile
    from concourse import mybir

    _patch_bir_wait_split()

    f32 = mybir.dt.float32
    bf16 = mybir.dt.bfloat16
    AL = mybir.AluOpType
    AF = mybir.ActivationFunctionType

    nc = bass.Bass("TRN2", target_bir_lowering=False, debug=False)
    yt = nc.dram_tensor("yt", [NPC, L], f32, kind="ExternalInput").ap()
    yp = nc.dram_tensor("yp", [NPC, L], f32, kind="ExternalInput").ap()
    yd = nc.dram_tensor("yd", [NPC, L], f32, kind="ExternalInput").ap()
    b0 = nc.dram_tensor("b0", [L, L], f32, kind="ExternalInput").ap()
    p2_out = nc.dram_tensor("p2_out", [L, L], f32, kind="ExternalOutput").ap()
    misc_out = nc.dram_tensor("misc_out", [L, 3], f32, kind="ExternalOutput").ap()
    acc_out = nc.dram_tensor("acc_out", [L, NTILES], f32, kind="ExternalOutput").ap()

    with tile.TileContext(nc) as tc, ExitStack() as ctx:
        const = ctx.enter_context(tc.tile_pool(name="const", bufs=1))
        data = ctx.enter_context(tc.tile_pool(name="data", bufs=1))
        scrp = ctx.enter_context(tc.tile_pool(name="scr", bufs=2))

        t_b0 = const.tile([L, L], f32)
        nc.sync.dma_start(t_b0[:], b0)
        t_b0bf = const.tile([L, L], bf16)
        nc.scalar.copy(t_b0bf[:], t_b0[:])
        ones = const.tile([L, 1], f32)
        nc.vector.memset(ones[:], 1.0)
        acc = const.tile([L, NTILES], f32)

        per = []
        for ch in range(NCH):
            rows = slice(ch * L, (ch + 1) * L)
            c = {}
            t_yt = data.tile([L, L], f32, tag=f"yt{ch}")
            t_yp = data.tile([L, L], f32, tag=f"yp{ch}")
            t_yd = data.tile([L, L], f32, tag=f"yd{ch}")
            nc.sync.dma_start(t_yt[:], yt[rows, :])
            nc.sync.dma_start(t_yp[:], yp[rows, :])
            nc.sync.dma_start(t_yd[:], yd[rows, :])

            d = data.tile([L, L], f32, tag=f"d{ch}")
            nc.vector.tensor_sub(d[:], t_yt[:], t_yp[:])
            absyd = data.tile([L, L], f32, tag=f"absyd{ch}")
            nc.scalar.activation(absyd[:], t_yd[:], AF.Abs)
            f = data.tile([L, L], f32, tag=f"f{ch}")
            nc.vector.tensor_scalar(out=f[:], in0=absyd[:], scalar1=FGV,
                                    scalar2=None, op0=AL.is_le)
            m = data.tile([L, L], f32, tag=f"m{ch}")
            nc.vector.tensor_tensor(out=m[:], in0=d[:], in1=f[:], op=AL.mult)
            # ACT-engine side computations
            h = data.tile([L, L], f32, tag=f"h{ch}")
            nc.scalar.square(h[:], m[:])
            mneg2 = data.tile([L, L], f32, tag=f"mneg2{ch}")
            nc.scalar.mul(mneg2[:], m[:], -2.0)
            absm = data.tile([L, L], f32, tag=f"absm{ch}")
            nc.scalar.activation(absm[:], m[:], AF.Abs)
            m_bf = data.tile([L, L], bf16, tag=f"mbf{ch}")
            nc.scalar.copy(m_bf[:], m[:])
            f_bf = data.tile([L, L], bf16, tag=f"fbf{ch}")
            nc.scalar.copy(f_bf[:], f[:])
            fneg_bf = data.tile([L, L], bf16, tag=f"fnbf{ch}")
            nc.scalar.mul(fneg_bf[:], f[:], -1.0)

            # flat 2-partition operand tiles (base partition 0 for K=2 matmuls)
            ilt = data.tile([2, L * L], bf16, tag=f"ilt{ch}")
            fmt = data.tile([2, L * L], bf16, tag=f"fmt{ch}")
            ilt_v = ilt[:].rearrange("p (e f) -> p e f", f=L)
            fmt_v = fmt[:].rearrange("p (e f) -> p e f", f=L)
            nc.sync.dma_start(ilt_v[0:1, :, :], m_bf[:])
            nc.sync.dma_start(ilt_v[1:2, :, :], fneg_bf[:])
            nc.sync.dma_start(fmt_v[0:1, :, :], f_bf[:])
            nc.sync.dma_start(fmt_v[1:2, :, :], m_bf[:])
            c.update(f=f, m=m, h=h, mneg2=mneg2, absm=absm, ilt=ilt, fmt=fmt)
            per.append(c)

        # p=2 factored term and lag-0/mean-f reductions (own PSUM scope,
        # closed before the X loop so the X pool gets all 8 banks)
        with tc.tile_pool(name="pst", bufs=1, space="PSUM") as pst:
            p2 = pst.tile([L, L], f32)
            steps = []
            for ch in range(NCH):
                c = per[ch]
                steps += [(c["h"], c["f"]), (c["f"], c["h"]), (c["m"], c["mneg2"])]
            for si, (lh, rh) in enumerate(steps):
                nc.tensor.matmul(p2[:], lhsT=lh[:], rhs=rh[:],
                                 start=(si == 0), stop=(si == len(steps) - 1))
            misc = pst.tile([L, 3], f32)
            for col, key in enumerate(["absm", "h", "f"]):
                for ch in range(NCH):
                    nc.tensor.matmul(misc[:, col:col + 1], lhsT=per[ch][key][:],
                                     rhs=ones[:], start=(ch == 0), stop=(ch == NCH - 1))
            p2_sb = data.tile([L, L], f32)
            nc.scalar.copy(p2_sb[:], p2[:])
            misc_sb = data.tile([L, 3], f32)
            nc.scalar.copy(misc_sb[:], misc[:])
        nc.sync.dma_start(p2_out, p2_sb[:])
        nc.sync.dma_start(misc_out, misc_sb[:])

        # main pairwise-abs loop. DVE is the bottleneck (fused relu+weight+
        # accum at 1 elem/lane/cycle fp32), so route some tiles through
        # ACT-Relu -> bf16 SBUF, whose bf16 DVE consume runs at 2x.
        N_BF = 6  # tiles offloaded to the ACT+bf16 path
        b0b = t_b0[:].rearrange("p (o f) -> p o f", o=1).broadcast_to(
            [L, EX_PER_TILE, L])
        b0b_bf = t_b0bf[:].rearrange("p (o f) -> p o f", o=1).broadcast_to(
            [L, EX_PER_TILE, L])
        with tc.tile_pool(name="psx", bufs=2, space="PSUM") as psx:
            for t in range(NTILES):
                ch = t // TILES_PER_CH
                ilt, fmt = per[ch]["ilt"], per[ch]["fmt"]
                xps = psx.tile([L, EX_PER_TILE * L], f32, tag="xps")
                for e in range(EX_PER_TILE):
                    le = (t % TILES_PER_CH) * EX_PER_TILE + e
                    nc.tensor.matmul(
                        xps[:, e * L:(e + 1) * L],
                        lhsT=ilt[0:2, le * L:(le + 1) * L],
                        rhs=fmt[0:2, le * L:(le + 1) * L],
                        start=True, stop=True)
                if t >= NTILES - N_BF:
                    relu_bf = scrp.tile([L, EX_PER_TILE * L], bf16,
                                        tag="relu_bf")
                    nc.scalar.activation(relu_bf[:], xps[:], AF.Relu)
                    scr_bf = scrp.tile([L, EX_PER_TILE * L], bf16,
                                       tag="scr_bf")
                    nc.vector.scalar_tensor_tensor(
                        out=scr_bf[:].rearrange("p (e f) -> p e f", f=L),
                        in0=relu_bf[:].rearrange("p (e f) -> p e f", f=L),
                        scalar=1.0, in1=b0b_bf,
                        op0=AL.mult, op1=AL.mult,
                        accum_out=acc[:, t:t + 1])
                else:
                    scr = scrp.tile([L, EX_PER_TILE * L], f32, tag="scr")
                    nc.vector.scalar_tensor_tensor(
                        out=scr[:].rearrange("p (e f) -> p e f", f=L),
                        in0=xps[:].rearrange("p (e f) -> p e f", f=L),
                        scalar=0.0, in1=b0b,
                        op0=AL.max, op1=AL.mult,
                        accum_out=acc[:, t:t + 1])
        nc.sync.dma_start(acc_out, acc[:])

    _STATE["nc"] = nc
    return _STATE


def _shear_upper(w):
    """B[i,j] = w[i, j-i] for j>i else 0 (strict upper; lag-0 handled apart)."""
    b = np.zeros((L, L), np.float64)
    i, j = np.meshgrid(np.arange(L), np.arange(L), indexing="ij")
    sel = j > i
    b[sel] = w[i[sel], (j - i)[sel]]
    return b


def kernel(y_true, y_pred, y_diff, weights):
    from concourse.bass_utils import run_bass_kernel_spmd

    st = _STATE if _STATE else _build_state()
    nc = st["nc"]

    y_true = np.ascontiguousarray(np.asarray(y_true, np.float32))
    y_pred = np.ascontiguousarray(np.asarray(y_pred, np.float32))
    y_diff = np.ascontiguousarray(np.asarray(y_diff, np.float32))
    w = np.asarray(weights, np.float64)
    b0u = _shear_upper(w[0])
    b1u = _shear_upper(w[1])
    # X_n is antisymmetric, so sum B0u .* |X| == sum (B0u+B0u^T) .* relu(X);
    # stock walrus lacks an abs ALU op, relu (max 0) is supported.
    b0_f32 = np.ascontiguousarray((b0u + b0u.T).astype(np.float32))

    in_maps = []
    for c in range(NCORES):
        rows = slice(c * NPC, (c + 1) * NPC)
        in_maps.append({
            "yt": y_true[rows], "yp": y_pred[rows], "yd": y_diff[rows],
            "b0": b0_f32,
        })
    _STATE["last_in_maps"] = in_maps
    res = run_bass_kernel_spmd(nc, in_maps, list(range(NCORES))).results

    p2 = np.zeros((L, L), np.float64)
    misc = np.zeros((L, 3), np.float64)
    pair1 = 0.0
    for c in range(NCORES):
        p2 += res[c]["p2_out"].astype(np.float64)
        misc += res[c]["misc_out"].astype(np.float64)
        pair1 += float(res[c]["acc_out"].astype(np.float64).sum())

    loss_num = (
        pair1
        + float((b1u * p2).sum())
        + float((w[0][:, 0] * misc[:, 0]).sum())
        + float((w[1][:, 0] * misc[:, 1]).sum())
    )
    sumf = float(misc[:, 2].sum())
    mean_f = sumf / (N * L)
    loss = loss_num / L / (N * mean_f)
    return np.float32(loss)


def bench_exec_ns(iters=300, warm=20):
    """Measure per-execution device time by looping the PJRT executable.

    All outputs are fully rewritten by the kernel, so the previous
    iteration's outputs can be donated as the next call's output buffers;
    inputs stay device-resident. Async dispatch queues executions
    back-to-back; the slope over the iteration count is the NEFF time
    (upper bound: includes any per-call dispatch the queue can't hide).
    """
    import time
    import jax
    import numpy as np
    from jax.sharding import Mesh, PartitionSpec, NamedSharding
    import concourse.bass2jax as b2j
    from concourse import mybir

    try:
        from jax.experimental.shard_map import shard_map
    except ImportError:
        from jax.shard_map import shard_map

    st = _STATE if _STATE else _build_state()
    nc = st["nc"]
    in_maps = st.get("last_in_maps")
    assert in_maps is not None, "call kernel() first"
    b2j.install_neuronx_cc_hook()

    partition_name = (nc.partition_id_tensor.name
                      if nc.partition_id_tensor else None)
    in_names, out_names, out_avals, zero_outs = [], [], [], []
    for alloc in nc.m.functions[0].allocations:
        if not isinstance(alloc, mybir.MemoryLocationSet):
            continue
        name = alloc.memorylocations[0].name
        if alloc.kind == "ExternalInput":
            if name != partition_name:
                in_names.append(name)
        elif alloc.kind == "ExternalOutput":
            shape = tuple(alloc.tensor_shape)
            dtype = mybir.dt.np(alloc.dtype)
            out_names.append(name)
            out_avals.append(jax.core.ShapedArray(shape, dtype))
            zero_outs.append(np.zeros(shape, dtype))
    n_params = len(in_names)
    n_outs = len(out_avals)
    all_in_names = list(in_names) + out_names + (
        [partition_name] if partition_name else [])

    def _body(*args):
        operands = list(args)
        if partition_name is not None:
            operands.append(b2j.partition_id_tensor())
        return tuple(b2j._bass_exec_p.bind(
            *operands, out_avals=tuple(out_avals),
            in_names=tuple(all_in_names), out_names=tuple(out_names),
            lowering_input_output_aliases=(), sim_require_finite=True,
            sim_require_nnan=True, nc=nc))

    devices = jax.devices()[:NCORES]
    mesh = Mesh(np.asarray(devices), ("core",))
    donate = tuple(range(n_params, n_params + n_outs))
    sharded = jax.jit(
        shard_map(_body, mesh=mesh,
                  in_specs=(PartitionSpec("core"),) * (n_params + n_outs),
                  out_specs=(PartitionSpec("core"),) * n_outs,
                  check_rep=False),
        donate_argnums=donate, keep_unused=True)

    sh = NamedSharding(mesh, PartitionSpec("core"))
    concat_in = [
        jax.device_put(
            np.concatenate([np.asarray(in_maps[c][nm]) for c in range(NCORES)],
                           axis=0), sh)
        for nm in in_names]
    outs = tuple(
        jax.device_put(np.zeros((NCORES * z.shape[0], *z.shape[1:]), z.dtype),
                       sh) for z in zero_outs)

    def loop(k):
        nonlocal outs
        t0 = time.perf_counter()
        for _ in range(k):
            outs = sharded(*concat_in, *outs)
        jax.block_until_ready(outs)
        return time.perf_counter() - t0

    loop(warm)
    t_small = loop(iters // 3)
    t_big = loop(iters)
    per_iter = (t_big - t_small) / (iters - iters // 3)
    return int(per_iter * 1e9)


def profile_exec_ns(tmpdir=None):
    """Re-run the last kernel invocation with NTFF tracing; return exec ns."""
    from concourse.bass_utils import run_bass_kernel_spmd

    st = _STATE if _STATE else _build_state()
    nc = st["nc"]
    in_maps = st.get("last_in_maps")
    assert in_maps is not None, "call kernel() first"
    if tmpdir is None:
        tmpdir = os.path.join(os.getcwd(), "trace_out")
        os.makedirs(tmpdir, exist_ok=True)
    r = run_bass_kernel_spmd(nc, in_maps, list(range(NCORES)), trace=True,
                             tmpdir=tmpdir)
    _STATE["last_profile"] = r
    return r.exec_time_ns



# revision 2
# speedup vs baseline: 13.4673x; 13.4673x over previous
"""Trainium2 Bass kernel for the generalized filtered pairwise loss.

Math (reference semantics, N=2048 examples, L=128 positions, p in {1,2}):
  d = y_true - y_pred;  f = 1{|y_diff| <= 2};  m = d*f;  h = m^2
  lag-0 term:   sum_{n,i} W0[i,0]*|m_i| + W1[i,0]*h_i
  lag-k term (j=i+k<L, k>0), with B_p[i,j] = W_p[i, j-i]:
    p=1: sum_{n,i<j} B0[i,j] * |m_i f_j - f_i m_j|   (pairwise, needs abs)
    p=2: <B1, H^T F + F^T H - 2 M^T M>               (factors into matmuls)
  loss = (sum of terms) / L / (N * mean(f))

Device strategy (8 cores, data-parallel over examples, 256/core):
  - per example e: X_e = m_e f_e^T - f_e m_e^T via one K=2 TensorE matmul
    from fp8e4 (e4m3) flat operand tiles (numpy-simulated p1 rel err
    7e-4 vs the 2e-2 tolerance; fp8 halves the flat-fill DMA bytes)
  - chunk A operand pair at partition base 0, chunk B at base 64:
    single-partition flat-fill DMA writes land on two different SDMA
    engines; fills are split across the sync and scalar HWDGE rings
  - X consumed via relu identity (X antisymmetric => sum B0u.*|X| equals
    sum (B0u+B0u^T).*relu(X)): fused DVE scalar_tensor_tensor with
    accum; 10 of 16 tiles are offloaded through ACT-Relu->bf16 so their
    DVE pass runs at 2x (balances ACT vs DVE)
  - p=2 + lag-0 + sum(f) reductions via a handful of K=128 matmuls
  - both 128-example chunks are prepped in merged [128, 256] tiles
    (halves elementwise op count)
  - small per-core partials DMA'd out; host combines in float64
"""


import numpy as np
from contextlib import ExitStack

N, L = 2048, 128
NCORES = 8
NPC = N // NCORES            # 256 examples per core
NCH = 2
EX_PER_TILE = 16
NTILES = NPC // EX_PER_TILE  # 16
TILES_PER_CH = NTILES // NCH
FGV = 2.0
N_BF = 10
X_MM_PER_TILE = EX_PER_TILE  # ablation: fewer matmuls per tile
DO_CONSUME = True           # ablation: skip the DVE/ACT consume

_STATE: dict = {}


def _patch_bir_wait_split():
    import json
    import concourse.bass_utils as bu
    import concourse.bass2jax as b2j

    if getattr(bu, "_wait_split_patched", False):
        return
    orig = bu.compile_bir_kernel

    def _split(bir_str):
        d = json.loads(bir_str)
        changed = False
        ctr = 0
        for fn in d.get("functions", []):
            for bb in fn.get("blocks", []):
                out = []
                for inst in bb.get("instructions", []):
                    si = inst.get("sync_info")
                    waits = (si or {}).get("on_wait") or []
                    if len(waits) > 1:
                        changed = True
                        for w in waits[:-1]:
                            ctr += 1
                            out.append({
                                "debug": inst.get("debug", 0),
                                "engine": inst["engine"],
                                "ins": [], "outs": [],
                                "name": f"{inst['name']}-ws{ctr}",
                                "opcode": "NoOp",
                                "sync_info": {"on_update": [], "on_wait": [w]},
                                "text_hint": "wait_split",
                            })
                        si["on_wait"] = [waits[-1]]
                    out.append(inst)
                bb["instructions"] = out
        if not changed:
            return bir_str
        return json.dumps(d).encode()

    def wrapper(bir_str, *args, **kwargs):
        return orig(_split(bir_str), *args, **kwargs)

    bu.compile_bir_kernel = wrapper
    b2j.compile_bir_kernel = wrapper
    bu._wait_split_patched = True


def build_body(nc, tc, mybir, rep, aps):
    """One kernel body. aps = (yt, yp, yd, b0, p2_out, misc_out, acc_out)."""
    yt, yp, yd, b0, p2_out, misc_out, acc_out = aps
    f32 = mybir.dt.float32
    bf16 = mybir.dt.bfloat16
    f8 = mybir.dt.float8e4
    AL = mybir.AluOpType
    AF = mybir.ActivationFunctionType
    W = L * NCH  # 256: merged free dim

    with ExitStack() as ctx:
        const = ctx.enter_context(tc.tile_pool(name=f"const{rep}", bufs=1))
        data = ctx.enter_context(tc.tile_pool(name=f"data{rep}", bufs=1))
        scrp = ctx.enter_context(tc.tile_pool(name=f"scr{rep}", bufs=2))

        t_b0 = const.tile([L, L], f32)
        nc.sync.dma_start(t_b0[:], b0)
        t_b0bf = const.tile([L, L], bf16)
        nc.scalar.copy(t_b0bf[:], t_b0[:])
        ones = const.tile([L, 1], f32)
        nc.vector.memset(ones[:], 1.0)
        acc = const.tile([L, NTILES], f32)

        # merged [128, 256] input tiles: cols [128ch:128ch+128] = chunk ch
        t_yt = data.tile([L, W], f32)
        t_yp = data.tile([L, W], f32)
        t_yd = data.tile([L, W], f32)
        for ch in range(NCH):
            rows = slice(ch * L, (ch + 1) * L)
            cols = slice(ch * L, (ch + 1) * L)
            nc.sync.dma_start(t_yt[:, cols], yt[rows, :])
            nc.sync.dma_start(t_yp[:, cols], yp[rows, :])
            nc.sync.dma_start(t_yd[:, cols], yd[rows, :])

        d = data.tile([L, W], f32)
        nc.vector.tensor_sub(d[:], t_yt[:], t_yp[:])
        absyd = data.tile([L, W], f32)
        nc.scalar.activation(absyd[:], t_yd[:], AF.Abs)
        f = data.tile([L, W], f32)
        nc.vector.tensor_scalar(out=f[:], in0=absyd[:], scalar1=FGV,
                                scalar2=None, op0=AL.is_le)
        m = data.tile([L, W], f32)
        nc.vector.tensor_tensor(out=m[:], in0=d[:], in1=f[:], op=AL.mult)
        h = data.tile([L, W], f32)
        nc.scalar.square(h[:], m[:])
        mneg2 = data.tile([L, W], f32)
        nc.scalar.mul(mneg2[:], m[:], -2.0)
        absm = data.tile([L, W], f32)
        nc.scalar.activation(absm[:], m[:], AF.Abs)

        # fp8 casts for the X-matmul operands
        m8 = data.tile([L, W], f8)
        nc.scalar.copy(m8[:], m[:])
        fn8 = data.tile([L, W], f8)
        nc.scalar.mul(fn8[:], f[:], -1.0)
        f8t = data.tile([L, W], f8)
        nc.scalar.copy(f8t[:], f[:])

        # flat operand tiles: lhs rows (m, -f), rhs rows (f, m). matmul
        # needs lhsT/rhs at the SAME base partition; chunk A pair at base
        # 0 (SDMA engine 0), chunk B pair at base 64 (engine 1), fills
        # split across the sync/scalar HWDGE rings.
        lhsA = data.tile([2, L * L], f8)
        rhsA = data.tile([2, L * L], f8)
        lhsB_full = data.tile([66, L * L], f8)
        rhsB_full = data.tile([66, L * L], f8)
        lhsB = lhsB_full[64:66, :]
        rhsB = rhsB_full[64:66, :]

        def fill(dst2, ch, srcs, eng):
            dv = dst2.rearrange("p (e f) -> p e f", f=L)
            cols = slice(ch * L, (ch + 1) * L)
            for r, src in enumerate(srcs):
                eng.dma_start(
                    dv[r:r + 1, :, :],
                    src[:, cols].rearrange("e (o f) -> e o f", o=1))

        fill(lhsA[:], 0, (m8, fn8), nc.sync)
        fill(rhsA[:], 0, (f8t, m8), nc.scalar)
        fill(lhsB, 1, (m8, fn8), nc.sync)
        fill(rhsB, 1, (f8t, m8), nc.scalar)

        # p=2 factored term + lag-0/sum-f reductions
        with tc.tile_pool(name=f"pst{rep}", bufs=1, space="PSUM") as pst:
            p2 = pst.tile([L, L], f32)
            steps = []
            for ch in range(NCH):
                cs = slice(ch * L, (ch + 1) * L)
                steps += [(h, f, cs), (f, h, cs), (m, mneg2, cs)]
            for si, (lh, rh, cs) in enumerate(steps):
                nc.tensor.matmul(p2[:], lhsT=lh[:, cs], rhs=rh[:, cs],
                                 start=(si == 0), stop=(si == len(steps) - 1))
            misc = pst.tile([L, 3], f32)
            for col, src in enumerate([absm, h, f]):
                for ch in range(NCH):
                    cs = slice(ch * L, (ch + 1) * L)
                    nc.tensor.matmul(misc[:, col:col + 1], lhsT=src[:, cs],
                                     rhs=ones[:], start=(ch == 0),
                                     stop=(ch == NCH - 1))
            p2_sb = data.tile([L, L], f32)
            nc.scalar.copy(p2_sb[:], p2[:])
            misc_sb = data.tile([L, 3], f32)
            nc.scalar.copy(misc_sb[:], misc[:])
        nc.sync.dma_start(p2_out, p2_sb[:])
        nc.sync.dma_start(misc_out, misc_sb[:])

        # pairwise-abs loop: X_e via K=2 fp8 matmuls; consume via DVE STT
        # (relu+weight+accum). N_BF tiles take ACT-relu->bf16 (DVE 2x).
        b0b = t_b0[:].rearrange("p (o f) -> p o f", o=1).broadcast_to(
            [L, EX_PER_TILE, L])
        b0b_bf = t_b0bf[:].rearrange("p (o f) -> p o f", o=1).broadcast_to(
            [L, EX_PER_TILE, L])
        # interleave bf16/fp32 tiles so ACT and DVE overlap
        flags = []
        nbf = nfp = 0
        for t in range(NTILES):
            if nbf * (NTILES - N_BF) <= nfp * N_BF and nbf < N_BF:
                flags.append(True); nbf += 1
            else:
                flags.append(False); nfp += 1
        with tc.tile_pool(name=f"psx{rep}", bufs=2, space="PSUM") as psx:
            for t in range(NTILES):
                ch = t // TILES_PER_CH
                lhs_t = lhsA[:] if ch == 0 else lhsB
                rhs_t = rhsA[:] if ch == 0 else rhsB
                xps = psx.tile([L, EX_PER_TILE * L], f32, tag="xps")
                for e in range(X_MM_PER_TILE):
                    le = (t % TILES_PER_CH) * EX_PER_TILE + e
                    nc.tensor.matmul(
                        xps[:, e * L:(e + 1) * L],
                        lhsT=lhs_t[0:2, le * L:(le + 1) * L],
                        rhs=rhs_t[0:2, le * L:(le + 1) * L],
                        start=True, stop=True)
                for e in range(X_MM_PER_TILE, EX_PER_TILE):
                    nc.tensor.matmul(
                        xps[:, e * L:(e + 1) * L],
                        lhsT=lhs_t[0:2, 0:L],
                        rhs=rhs_t[0:2, 0:L],
                        start=True, stop=True) if False else None
                if not DO_CONSUME:
                    small = scrp.tile([L, EX_PER_TILE], f32, tag="small")
                    nc.vector.scalar_tensor_tensor(
                        out=small[:],
                        in0=xps[:, 0:EX_PER_TILE],
                        scalar=0.0, in1=t_b0[:, 0:EX_PER_TILE],
                        op0=AL.max, op1=AL.mult,
                        accum_out=acc[:, t:t + 1])
                elif flags[t]:
                    relu_bf = scrp.tile([L, EX_PER_TILE * L], bf16,
                                        tag="relu_bf")
                    nc.scalar.activation(relu_bf[:], xps[:], AF.Relu)
                    scr_bf = scrp.tile([L, EX_PER_TILE * L], bf16,
                                       tag="scr_bf")
                    nc.vector.scalar_tensor_tensor(
                        out=scr_bf[:].rearrange("p (e f) -> p e f", f=L),
                        in0=relu_bf[:].rearrange("p (e f) -> p e f", f=L),
                        scalar=1.0, in1=b0b_bf,
                        op0=AL.mult, op1=AL.mult,
                        accum_out=acc[:, t:t + 1])
                else:
                    scr = scrp.tile([L, EX_PER_TILE * L], f32, tag="scr")
                    nc.vector.scalar_tensor_tensor(
                        out=scr[:].rearrange("p (e f) -> p e f", f=L),
                        in0=xps[:].rearrange("p (e f) -> p e f", f=L),
                        scalar=0.0, in1=b0b,
                        op0=AL.max, op1=AL.mult,
                        accum_out=acc[:, t:t + 1])
        nc.sync.dma_start(acc_out, acc[:])


def _declare_io(nc, mybir):
    f32 = mybir.dt.float32
    yt = nc.dram_tensor("yt", [NPC, L], f32, kind="ExternalInput").ap()
    yp = nc.dram_tensor("yp", [NPC, L], f32, kind="ExternalInput").ap()
    yd = nc.dram_tensor("yd", [NPC, L], f32, kind="ExternalInput").ap()
    b0 = nc.dram_tensor("b0", [L, L], f32, kind="ExternalInput").ap()
    p2_out = nc.dram_tensor("p2_out", [L, L], f32, kind="ExternalOutput").ap()
    misc_out = nc.dram_tensor("misc_out", [L, 3], f32,
                              kind="ExternalOutput").ap()
    acc_out = nc.dram_tensor("acc_out", [L, NTILES], f32,
                             kind="ExternalOutput").ap()
    return yt, yp, yd, b0, p2_out, misc_out, acc_out


def build(loop_iters=None, unroll=1):
    """loop_iters=None: single body (production kernel).
    Otherwise: For_i(0, loop_iters) { unroll x body } for timing."""
    import concourse.bass as bass
    import concourse.tile as tile
    from concourse import mybir

    _patch_bir_wait_split()
    nc = bass.Bass("TRN2", target_bir_lowering=False, debug=False)
    aps = _declare_io(nc, mybir)
    with tile.TileContext(nc) as tc:
        if loop_iters is None:
            build_body(nc, tc, mybir, 0, aps)
        else:
            with tc.For_i(0, loop_iters, 1):
                for u in range(unroll):
                    build_body(nc, tc, mybir, u, aps)
    return nc


def _shear_upper(w):
    b = np.zeros((L, L), np.float64)
    i, j = np.meshgrid(np.arange(L), np.arange(L), indexing="ij")
    sel = j > i
    b[sel] = w[i[sel], (j - i)[sel]]
    return b


def combine(results, weights):
    """Host-side combine of per-core partials (float64)."""
    w = np.asarray(weights, np.float64)
    b0u = _shear_upper(w[0])
    b1u = _shear_upper(w[1])
    p2 = np.zeros((L, L), np.float64)
    misc = np.zeros((L, 3), np.float64)
    pair1 = 0.0
    for c in range(len(results)):
        p2 += results[c]["p2_out"].astype(np.float64)
        misc += results[c]["misc_out"].astype(np.float64)
        pair1 += float(results[c]["acc_out"].astype(np.float64).sum())
    loss_num = (
        pair1
        + float((b1u * p2).sum())
        + float((w[0][:, 0] * misc[:, 0]).sum())
        + float((w[1][:, 0] * misc[:, 1]).sum())
    )
    sumf = float(misc[:, 2].sum())
    mean_f = sumf / (N * L)
    return np.float32(loss_num / L / (N * mean_f))


def make_in_maps(y_true, y_pred, y_diff, weights):
    y_true = np.ascontiguousarray(np.asarray(y_true, np.float32))
    y_pred = np.ascontiguousarray(np.asarray(y_pred, np.float32))
    y_diff = np.ascontiguousarray(np.asarray(y_diff, np.float32))
    w = np.asarray(weights, np.float64)
    b0u = _shear_upper(w[0])
    b0_f32 = np.ascontiguousarray((b0u + b0u.T).astype(np.float32))
    in_maps = []
    for c in range(NCORES):
        rows = slice(c * NPC, (c + 1) * NPC)
        in_maps.append({
            "yt": y_true[rows], "yp": y_pred[rows], "yd": y_diff[rows],
            "b0": b0_f32,
        })
    return in_maps


def kernel(y_true, y_pred, y_diff, weights):
    from concourse.bass_utils import run_bass_kernel_spmd

    if "nc" not in _STATE:
        _STATE["nc"] = build()
    nc = _STATE["nc"]
    in_maps = make_in_maps(y_true, y_pred, y_diff, weights)
    _STATE["last_in_maps"] = in_maps
    res = run_bass_kernel_spmd(nc, in_maps, list(range(NCORES))).results
    return combine(res, weights)


# ---------------------------------------------------------------------------
# Device-time measurement.
#
# Per-call axon dispatch overhead is ~0.4-1.5 ms with +-50 us drift and does
# not pipeline, so naive loop timing cannot resolve the kernel's device time.
# Instead the body is repeated inside one NEFF with a Tile For_i hardware
# loop; two NEFFs differing ONLY in bodies per iteration (unroll 2 vs 1,
# same loop count, same back-edge barriers) are timed in interleaved
# batches. The difference isolates the steady-state device time of one
# kernel body (all 8 cores run concurrently; per-core partials as in
# kernel()).
# ---------------------------------------------------------------------------

def _make_runner(nc, in_maps, ncores=NCORES):
    import time
    import jax
    from jax.sharding import Mesh, PartitionSpec, NamedSharding
    import concourse.bass2jax as b2j
    from concourse import mybir
    try:
        from jax.experimental.shard_map import shard_map
    except ImportError:
        from jax.shard_map import shard_map

    b2j.install_neuronx_cc_hook()
    partition_name = (nc.partition_id_tensor.name
                      if nc.partition_id_tensor else None)
    in_names, out_names, out_avals, zero_outs = [], [], [], []
    for alloc in nc.m.functions[0].allocations:
        if not isinstance(alloc, mybir.MemoryLocationSet):
            continue
        name = alloc.memorylocations[0].name
        if alloc.kind == "ExternalInput":
            if name != partition_name:
                in_names.append(name)
        elif alloc.kind == "ExternalOutput":
            shape = tuple(alloc.tensor_shape)
            dtype = mybir.dt.np(alloc.dtype)
            out_names.append(name)
            out_avals.append(jax.core.ShapedArray(shape, dtype))
            zero_outs.append(np.zeros(shape, dtype))
    n_params = len(in_names)
    n_outs = len(out_avals)
    all_in_names = list(in_names) + out_names + (
        [partition_name] if partition_name else [])

    def _body(*args):
        operands = list(args)
        if partition_name is not None:
            operands.append(b2j.partition_id_tensor())
        return tuple(b2j._bass_exec_p.bind(
            *operands, out_avals=tuple(out_avals),
            in_names=tuple(all_in_names), out_names=tuple(out_names),
            lowering_input_output_aliases=(), sim_require_finite=True,
            sim_require_nnan=True, nc=nc))

    devices = jax.devices()[:ncores]
    mesh = Mesh(np.asarray(devices), ("core",))
    donate = tuple(range(n_params, n_params + n_outs))
    sharded = jax.jit(
        shard_map(_body, mesh=mesh,
                  in_specs=(PartitionSpec("core"),) * (n_params + n_outs),
                  out_specs=(PartitionSpec("core"),) * n_outs,
                  check_rep=False),
        donate_argnums=donate, keep_unused=True)

    sh = NamedSharding(mesh, PartitionSpec("core"))
    concat_in = [
        jax.device_put(
            np.concatenate([np.asarray(in_maps[c][nm]) for c in range(ncores)],
                           axis=0), sh)
        for nm in in_names]
    state = {"outs": tuple(
        jax.device_put(np.zeros((ncores * z.shape[0], *z.shape[1:]), z.dtype),
                       sh) for z in zero_outs)}

    def run_batch(k):
        import time as _t
        import jax as _jax
        outs = state["outs"]
        t0 = _t.perf_counter()
        for _ in range(k):
            outs = sharded(*concat_in, *outs)
        _jax.block_until_ready(outs)
        state["outs"] = outs
        return _t.perf_counter() - t0

    return run_batch


def hw_exec_ns(r_a=2, r_b=27, unroll=4, batch=20, rounds=8, warm=8):
    """Device ns per kernel body: A/B slope between two For_i NEFFs that
    differ only in loop count (r_a vs r_b iterations of `unroll` bodies).
    The per-call dispatch overhead cancels in the interleaved difference;
    the For_i back-edge barrier (measured ~0 with an empty body) is
    amortized over `unroll` bodies."""
    in_maps = _STATE.get("last_in_maps")
    assert in_maps is not None, "call kernel() first"
    nc1 = build(loop_iters=r_a, unroll=unroll)
    nc2 = build(loop_iters=r_b, unroll=unroll)
    r1 = _make_runner(nc1, in_maps)
    r2 = _make_runner(nc2, in_maps)
    r1(warm)
    r2(warm)
    diffs = []
    for _ in range(rounds):
        t1 = r1(batch) / batch
        t2 = r2(batch) / batch
        diffs.append(t2 - t1)
    med = float(np.median(diffs))
    return int(med / ((r_b - r_a) * unroll) * 1e9)


# revision 3
# speedup vs baseline: 14.1448x; 1.0503x over previous
"""Trainium2 Bass kernel for the generalized filtered pairwise loss.

Math (reference semantics, N=2048 examples, L=128 positions, p in {1,2}):
  d = y_true - y_pred;  f = 1{|y_diff| <= 2};  m = d*f;  h = m^2
  lag-0 term:   sum_{n,i} W0[i,0]*|m_i| + W1[i,0]*h_i
  lag-k term (j=i+k<L, k>0), with B_p[i,j] = W_p[i, j-i]:
    p=1: sum_{n,i<j} B0[i,j] * |m_i f_j - f_i m_j|   (pairwise, needs abs)
    p=2: <B1, H^T F + F^T H - 2 M^T M>               (factors into matmuls)
  loss = (sum of terms) / L / (N * mean(f))

Device strategy (8 cores, data-parallel over examples, 256/core):
  - per example e: X_e = m_e f_e^T - f_e m_e^T via one K=2 TensorE matmul
    from fp8e4 (e4m3) flat operand tiles (numpy-simulated p1 rel err
    7e-4 vs the 2e-2 tolerance; fp8 halves the flat-fill DMA bytes)
  - chunk A operand pair at partition base 0, chunk B at base 64:
    single-partition flat-fill DMA writes land on two different SDMA
    engines; fills are split across the sync and scalar HWDGE rings
  - X consumed via relu identity (X antisymmetric => sum B0u.*|X| equals
    sum (B0u+B0u^T).*relu(X)): fused DVE scalar_tensor_tensor with
    accum; 10 of 16 tiles are offloaded through ACT-Relu->bf16 so their
    DVE pass runs at 2x (balances ACT vs DVE)
  - p=2 + lag-0 + sum(f) reductions via a handful of K=128 matmuls
  - both 128-example chunks are prepped in merged [128, 256] tiles
    (halves elementwise op count)
  - small per-core partials DMA'd out; host combines in float64
"""


import numpy as np
from contextlib import ExitStack

N, L = 2048, 128
NCORES = 8
NPC = N // NCORES            # 256 examples per core
NCH = 2
EX_PER_TILE = 16
NTILES = NPC // EX_PER_TILE  # 16
TILES_PER_CH = NTILES // NCH
FGV = 2.0
N_BF = 10
X_MM_PER_TILE = EX_PER_TILE  # ablation: fewer matmuls per tile
DO_CONSUME = True           # ablation: skip the DVE/ACT consume

_STATE: dict = {}


def _patch_bir_wait_split():
    import json
    import concourse.bass_utils as bu
    import concourse.bass2jax as b2j

    if getattr(bu, "_wait_split_patched", False):
        return
    orig = bu.compile_bir_kernel

    def _split(bir_str):
        d = json.loads(bir_str)
        changed = False
        ctr = 0
        for fn in d.get("functions", []):
            for bb in fn.get("blocks", []):
                out = []
                for inst in bb.get("instructions", []):
                    si = inst.get("sync_info")
                    waits = (si or {}).get("on_wait") or []
                    if len(waits) > 1:
                        changed = True
                        for w in waits[:-1]:
                            ctr += 1
                            out.append({
                                "debug": inst.get("debug", 0),
                                "engine": inst["engine"],
                                "ins": [], "outs": [],
                                "name": f"{inst['name']}-ws{ctr}",
                                "opcode": "NoOp",
                                "sync_info": {"on_update": [], "on_wait": [w]},
                                "text_hint": "wait_split",
                            })
                        si["on_wait"] = [waits[-1]]
                    out.append(inst)
                bb["instructions"] = out
        if not changed:
            return bir_str
        return json.dumps(d).encode()

    def wrapper(bir_str, *args, **kwargs):
        return orig(_split(bir_str), *args, **kwargs)

    bu.compile_bir_kernel = wrapper
    b2j.compile_bir_kernel = wrapper
    bu._wait_split_patched = True


def build_body(nc, tc, mybir, rep, aps):
    """One kernel body. aps = (yt, yp, yd, b0, p2_out, misc_out, acc_out)."""
    yt, yp, yd, b0, p2_out, misc_out, acc_out = aps
    f32 = mybir.dt.float32
    bf16 = mybir.dt.bfloat16
    f8 = mybir.dt.float8e4
    AL = mybir.AluOpType
    AF = mybir.ActivationFunctionType
    W = L * NCH  # 256: merged free dim

    with ExitStack() as ctx:
        const = ctx.enter_context(tc.tile_pool(name=f"const{rep}", bufs=1))
        data = ctx.enter_context(tc.tile_pool(name=f"data{rep}", bufs=1))
        scrp = ctx.enter_context(tc.tile_pool(name=f"scr{rep}", bufs=2))

        t_b0 = const.tile([L, L], f32)
        nc.sync.dma_start(t_b0[:], b0)
        t_b0bf = const.tile([L, L], bf16)
        nc.scalar.copy(t_b0bf[:], t_b0[:])
        ones = const.tile([L, 1], f32)
        nc.vector.memset(ones[:], 1.0)
        acc = const.tile([L, NTILES], f32)

        # merged [128, 256] input tiles: cols [128ch:128ch+128] = chunk ch
        t_yt = data.tile([L, W], f32)
        t_yp = data.tile([L, W], f32)
        t_yd = data.tile([L, W], f32)
        for ch in range(NCH):
            rows = slice(ch * L, (ch + 1) * L)
            cols = slice(ch * L, (ch + 1) * L)
            nc.sync.dma_start(t_yt[:, cols], yt[rows, :])
            nc.sync.dma_start(t_yp[:, cols], yp[rows, :])
            nc.sync.dma_start(t_yd[:, cols], yd[rows, :])

        d = data.tile([L, W], f32)
        nc.vector.tensor_sub(d[:], t_yt[:], t_yp[:])
        absyd = data.tile([L, W], f32)
        nc.scalar.activation(absyd[:], t_yd[:], AF.Abs)
        f = data.tile([L, W], f32)
        nc.vector.tensor_scalar(out=f[:], in0=absyd[:], scalar1=FGV,
                                scalar2=None, op0=AL.is_le)
        m = data.tile([L, W], f32)
        nc.vector.tensor_tensor(out=m[:], in0=d[:], in1=f[:], op=AL.mult)
        h = data.tile([L, W], f32)
        nc.scalar.square(h[:], m[:])
        mneg2 = data.tile([L, W], f32)
        nc.scalar.mul(mneg2[:], m[:], -2.0)
        absm = data.tile([L, W], f32)
        nc.scalar.activation(absm[:], m[:], AF.Abs)

        # fp8 casts for the X-matmul operands
        m8 = data.tile([L, W], f8)
        nc.scalar.copy(m8[:], m[:])
        fn8 = data.tile([L, W], f8)
        nc.scalar.mul(fn8[:], f[:], -1.0)
        f8t = data.tile([L, W], f8)
        nc.scalar.copy(f8t[:], f[:])

        # flat operand tiles: lhs rows (m, -f), rhs rows (f, m). matmul
        # needs lhsT/rhs at the SAME base partition; chunk A pair at base
        # 0 (SDMA engine 0), chunk B pair at base 64 (engine 1), fills
        # split across the sync/scalar HWDGE rings.
        lhsA = data.tile([2, L * L], f8)
        rhsA = data.tile([2, L * L], f8)
        lhsB_full = data.tile([66, L * L], f8)
        rhsB_full = data.tile([66, L * L], f8)
        lhsB = lhsB_full[64:66, :]
        rhsB = rhsB_full[64:66, :]

        def fill(dst2, ch, srcs, eng):
            dv = dst2.rearrange("p (e f) -> p e f", f=L)
            cols = slice(ch * L, (ch + 1) * L)
            for r, src in enumerate(srcs):
                eng.dma_start(
                    dv[r:r + 1, :, :],
                    src[:, cols].rearrange("e (o f) -> e o f", o=1))

        fill(lhsA[:], 0, (m8, fn8), nc.sync)
        fill(rhsA[:], 0, (f8t, m8), nc.scalar)
        fill(lhsB, 1, (m8, fn8), nc.sync)
        fill(rhsB, 1, (f8t, m8), nc.scalar)

        # p=2 factored term + lag-0/sum-f reductions
        with tc.tile_pool(name=f"pst{rep}", bufs=1, space="PSUM") as pst:
            p2 = pst.tile([L, L], f32)
            steps = []
            for ch in range(NCH):
                cs = slice(ch * L, (ch + 1) * L)
                steps += [(h, f, cs), (f, h, cs), (m, mneg2, cs)]
            for si, (lh, rh, cs) in enumerate(steps):
                nc.tensor.matmul(p2[:], lhsT=lh[:, cs], rhs=rh[:, cs],
                                 start=(si == 0), stop=(si == len(steps) - 1))
            misc = pst.tile([L, 3], f32)
            for col, src in enumerate([absm, h, f]):
                for ch in range(NCH):
                    cs = slice(ch * L, (ch + 1) * L)
                    nc.tensor.matmul(misc[:, col:col + 1], lhsT=src[:, cs],
                                     rhs=ones[:], start=(ch == 0),
                                     stop=(ch == NCH - 1))
            p2_sb = data.tile([L, L], f32)
            nc.scalar.copy(p2_sb[:], p2[:])
            misc_sb = data.tile([L, 3], f32)
            nc.scalar.copy(misc_sb[:], misc[:])
        nc.sync.dma_start(p2_out, p2_sb[:])
        nc.sync.dma_start(misc_out, misc_sb[:])

        # pairwise-abs loop: X_e via K=2 fp8 matmuls; consume via DVE STT
        # (relu+weight+accum). N_BF tiles take ACT-relu->bf16 (DVE 2x).
        b0b = t_b0[:].rearrange("p (o f) -> p o f", o=1).broadcast_to(
            [L, EX_PER_TILE, L])
        b0b_bf = t_b0bf[:].rearrange("p (o f) -> p o f", o=1).broadcast_to(
            [L, EX_PER_TILE, L])
        # interleave bf16/fp32 tiles so ACT and DVE overlap
        flags = []
        nbf = nfp = 0
        for t in range(NTILES):
            if nbf * (NTILES - N_BF) <= nfp * N_BF and nbf < N_BF:
                flags.append(True); nbf += 1
            else:
                flags.append(False); nfp += 1
        with tc.tile_pool(name=f"psx{rep}", bufs=2, space="PSUM") as psx:
            for t in range(NTILES):
                ch = t // TILES_PER_CH
                lhs_t = lhsA[:] if ch == 0 else lhsB
                rhs_t = rhsA[:] if ch == 0 else rhsB
                xps = psx.tile([L, EX_PER_TILE * L], f32, tag="xps")
                for e in range(X_MM_PER_TILE):
                    le = (t % TILES_PER_CH) * EX_PER_TILE + e
                    nc.tensor.matmul(
                        xps[:, e * L:(e + 1) * L],
                        lhsT=lhs_t[0:2, le * L:(le + 1) * L],
                        rhs=rhs_t[0:2, le * L:(le + 1) * L],
                        start=True, stop=True)
                for e in range(X_MM_PER_TILE, EX_PER_TILE):
                    nc.tensor.matmul(
                        xps[:, e * L:(e + 1) * L],
                        lhsT=lhs_t[0:2, 0:L],
                        rhs=rhs_t[0:2, 0:L],
                        start=True, stop=True) if False else None
                if not DO_CONSUME:
                    small = scrp.tile([L, EX_PER_TILE], f32, tag="small")
                    nc.vector.scalar_tensor_tensor(
                        out=small[:],
                        in0=xps[:, 0:EX_PER_TILE],
                        scalar=0.0, in1=t_b0[:, 0:EX_PER_TILE],
                        op0=AL.max, op1=AL.mult,
                        accum_out=acc[:, t:t + 1])
                elif flags[t]:
                    relu_bf = scrp.tile([L, EX_PER_TILE * L], bf16,
                                        tag="relu_bf")
                    nc.scalar.activation(relu_bf[:], xps[:], AF.Relu)
                    scr_bf = scrp.tile([L, EX_PER_TILE * L], bf16,
                                       tag="scr_bf")
                    nc.vector.scalar_tensor_tensor(
                        out=scr_bf[:].rearrange("p (e f) -> p e f", f=L),
                        in0=relu_bf[:].rearrange("p (e f) -> p e f", f=L),
                        scalar=1.0, in1=b0b_bf,
                        op0=AL.mult, op1=AL.mult,
                        accum_out=acc[:, t:t + 1])
                else:
                    scr = scrp.tile([L, EX_PER_TILE * L], f32, tag="scr")
                    nc.vector.scalar_tensor_tensor(
                        out=scr[:].rearrange("p (e f) -> p e f", f=L),
                        in0=xps[:].rearrange("p (e f) -> p e f", f=L),
                        scalar=0.0, in1=b0b,
                        op0=AL.max, op1=AL.mult,
                        accum_out=acc[:, t:t + 1])
        nc.sync.dma_start(acc_out, acc[:])


def _declare_io(nc, mybir):
    f32 = mybir.dt.float32
    yt = nc.dram_tensor("yt", [NPC, L], f32, kind="ExternalInput").ap()
    yp = nc.dram_tensor("yp", [NPC, L], f32, kind="ExternalInput").ap()
    yd = nc.dram_tensor("yd", [NPC, L], f32, kind="ExternalInput").ap()
    b0 = nc.dram_tensor("b0", [L, L], f32, kind="ExternalInput").ap()
    p2_out = nc.dram_tensor("p2_out", [L, L], f32, kind="ExternalOutput").ap()
    misc_out = nc.dram_tensor("misc_out", [L, 3], f32,
                              kind="ExternalOutput").ap()
    acc_out = nc.dram_tensor("acc_out", [L, NTILES], f32,
                             kind="ExternalOutput").ap()
    return yt, yp, yd, b0, p2_out, misc_out, acc_out


def build(loop_iters=None, unroll=1):
    """loop_iters=None: single body (production kernel).
    Otherwise: For_i(0, loop_iters) { unroll x body } for timing."""
    import concourse.bass as bass
    import concourse.tile as tile
    from concourse import mybir

    _patch_bir_wait_split()
    nc = bass.Bass("TRN2", target_bir_lowering=False, debug=False)
    aps = _declare_io(nc, mybir)
    with tile.TileContext(nc) as tc:
        if loop_iters is None:
            build_body(nc, tc, mybir, 0, aps)
        else:
            with tc.For_i(0, loop_iters, 1):
                for u in range(unroll):
                    build_body(nc, tc, mybir, u, aps)
    return nc


def _shear_upper(w):
    b = np.zeros((L, L), np.float64)
    i, j = np.meshgrid(np.arange(L), np.arange(L), indexing="ij")
    sel = j > i
    b[sel] = w[i[sel], (j - i)[sel]]
    return b


def combine(results, weights):
    """Host-side combine of per-core partials (float64)."""
    w = np.asarray(weights, np.float64)
    b0u = _shear_upper(w[0])
    b1u = _shear_upper(w[1])
    p2 = np.zeros((L, L), np.float64)
    misc = np.zeros((L, 3), np.float64)
    pair1 = 0.0
    for c in range(len(results)):
        p2 += results[c]["p2_out"].astype(np.float64)
        misc += results[c]["misc_out"].astype(np.float64)
        pair1 += float(results[c]["acc_out"].astype(np.float64).sum())
    loss_num = (
        pair1
        + float((b1u * p2).sum())
        + float((w[0][:, 0] * misc[:, 0]).sum())
        + float((w[1][:, 0] * misc[:, 1]).sum())
    )
    sumf = float(misc[:, 2].sum())
    mean_f = sumf / (N * L)
    return np.float32(loss_num / L / (N * mean_f))


def make_in_maps(y_true, y_pred, y_diff, weights):
    y_true = np.ascontiguousarray(np.asarray(y_true, np.float32))
    y_pred = np.ascontiguousarray(np.asarray(y_pred, np.float32))
    y_diff = np.ascontiguousarray(np.asarray(y_diff, np.float32))
    w = np.asarray(weights, np.float64)
    b0u = _shear_upper(w[0])
    b0_f32 = np.ascontiguousarray((b0u + b0u.T).astype(np.float32))
    in_maps = []
    for c in range(NCORES):
        rows = slice(c * NPC, (c + 1) * NPC)
        in_maps.append({
            "yt": y_true[rows], "yp": y_pred[rows], "yd": y_diff[rows],
            "b0": b0_f32,
        })
    return in_maps


def kernel(y_true, y_pred, y_diff, weights):
    from concourse.bass_utils import run_bass_kernel_spmd

    if "nc" not in _STATE:
        _STATE["nc"] = build()
    nc = _STATE["nc"]
    in_maps = make_in_maps(y_true, y_pred, y_diff, weights)
    _STATE["last_in_maps"] = in_maps
    res = run_bass_kernel_spmd(nc, in_maps, list(range(NCORES))).results
    return combine(res, weights)


# ---------------------------------------------------------------------------
# Device-time measurement.
#
# Per-call axon dispatch overhead is ~0.4-1.5 ms with +-50 us drift and does
# not pipeline, so naive loop timing cannot resolve the kernel's device time.
# Instead the body is repeated inside one NEFF with a Tile For_i hardware
# loop; two NEFFs differing ONLY in bodies per iteration (unroll 2 vs 1,
# same loop count, same back-edge barriers) are timed in interleaved
# batches. The difference isolates the steady-state device time of one
# kernel body (all 8 cores run concurrently; per-core partials as in
# kernel()).
# ---------------------------------------------------------------------------

def _make_runner(nc, in_maps, ncores=NCORES):
    import time
    import jax
    from jax.sharding import Mesh, PartitionSpec, NamedSharding
    import concourse.bass2jax as b2j
    from concourse import mybir
    try:
        from jax.experimental.shard_map import shard_map
    except ImportError:
        from jax.shard_map import shard_map

    b2j.install_neuronx_cc_hook()
    partition_name = (nc.partition_id_tensor.name
                      if nc.partition_id_tensor else None)
    in_names, out_names, out_avals, zero_outs = [], [], [], []
    for alloc in nc.m.functions[0].allocations:
        if not isinstance(alloc, mybir.MemoryLocationSet):
            continue
        name = alloc.memorylocations[0].name
        if alloc.kind == "ExternalInput":
            if name != partition_name:
                in_names.append(name)
        elif alloc.kind == "ExternalOutput":
            shape = tuple(alloc.tensor_shape)
            dtype = mybir.dt.np(alloc.dtype)
            out_names.append(name)
            out_avals.append(jax.core.ShapedArray(shape, dtype))
            zero_outs.append(np.zeros(shape, dtype))
    n_params = len(in_names)
    n_outs = len(out_avals)
    all_in_names = list(in_names) + out_names + (
        [partition_name] if partition_name else [])

    def _body(*args):
        operands = list(args)
        if partition_name is not None:
            operands.append(b2j.partition_id_tensor())
        return tuple(b2j._bass_exec_p.bind(
            *operands, out_avals=tuple(out_avals),
            in_names=tuple(all_in_names), out_names=tuple(out_names),
            lowering_input_output_aliases=(), sim_require_finite=True,
            sim_require_nnan=True, nc=nc))

    devices = jax.devices()[:ncores]
    mesh = Mesh(np.asarray(devices), ("core",))
    donate = tuple(range(n_params, n_params + n_outs))
    sharded = jax.jit(
        shard_map(_body, mesh=mesh,
                  in_specs=(PartitionSpec("core"),) * (n_params + n_outs),
                  out_specs=(PartitionSpec("core"),) * n_outs,
                  check_rep=False),
        donate_argnums=donate, keep_unused=True)

    sh = NamedSharding(mesh, PartitionSpec("core"))
    concat_in = [
        jax.device_put(
            np.concatenate([np.asarray(in_maps[c][nm]) for c in range(ncores)],
                           axis=0), sh)
        for nm in in_names]
    state = {"outs": tuple(
        jax.device_put(np.zeros((ncores * z.shape[0], *z.shape[1:]), z.dtype),
                       sh) for z in zero_outs)}

    def run_batch(k):
        import time as _t
        import jax as _jax
        outs = state["outs"]
        t0 = _t.perf_counter()
        for _ in range(k):
            outs = sharded(*concat_in, *outs)
        _jax.block_until_ready(outs)
        state["outs"] = outs
        return _t.perf_counter() - t0

    return run_batch


def hw_exec_ns(r_a=2, r_b=27, unroll=4, batch=20, rounds=12, warm=8):
    """Device ns per kernel body: A/B slope between two For_i NEFFs that
    differ only in loop count (r_a vs r_b iterations of `unroll` bodies).
    The per-call dispatch overhead cancels in the interleaved difference;
    the For_i back-edge barrier (measured ~0 with an empty body) is
    amortized over `unroll` bodies."""
    in_maps = _STATE.get("last_in_maps")
    assert in_maps is not None, "call kernel() first"
    nc1 = build(loop_iters=r_a, unroll=unroll)
    nc2 = build(loop_iters=r_b, unroll=unroll)
    r1 = _make_runner(nc1, in_maps)
    r2 = _make_runner(nc2, in_maps)
    r1(warm)
    r2(warm)
    diffs = []
    for i in range(rounds):
        if i % 2 == 0:
            t1 = r1(batch) / batch
            t2 = r2(batch) / batch
        else:
            t2 = r2(batch) / batch
            t1 = r1(batch) / batch
        diffs.append(t2 - t1)
    med = float(np.median(diffs))
    return int(med / ((r_b - r_a) * unroll) * 1e9)
